# revision 10
# baseline (speedup 1.0000x reference)
"""DCT channel attention kernel for Trainium2 (8 NeuronCores, data-parallel over batch).

Math (per image b, channel c):
  Y = DH @ X @ DW^T              (2D orthonormal DCT of the 64x64 spatial map)
  energy = |Y[0,0]| + sum(top4(|Y| excluding DC))
  attn = sigmoid(relu(energy @ w1 + b1) @ w2 + b2)
  out = x * attn[:, :, None, None]

V2 layout strategy per core (4 images):
  x streamed C-MAJOR: xc [128 = c%128, (cs, h, w)] — 16KB-contiguous DMA runs,
  full HBM rate both directions (the h-major layout measured 183 GB/s vs 351).
  DCT path uses an on-chip bf16 h-major copy xh [128=(b2,h), (c,w)] built by
  SBUF->SBUF DMA (+cast). M1 = per-channel-pair matmuls with x-slices as
  stationary bf16 weights (FWL), rhs = DH^T f32r -> A^T psum. M2 = stationary
  blockdiag(DW^T) f32r, rhs = A^T bf16 -> Y^T psum; ScalarE Abs-evicts bf16.
  Flatten |Y| to channel-major via SBUF->SBUF DMA, DVE Max8 top-8 scan,
  energy -> tiny MLP on PE with TRUE-channel-order w2 so attn emerges as a
  per-partition scalar column; DVE tensor_scalar (2x) multiplies xc in place.

Channel permutation: flat row q within group g of 128 channels maps to true
channel c = g*128 + 2*(q % 64) + (q // 64). Only w1 is permuted host-side.
"""

import numpy as np

B, C, H, W = 32, 256, 64, 64
NCORES = 8
BPC = B // NCORES  # images per core
RED = 4
CH = C // RED  # 64 hidden units

XH_MODE = "castx"  # "swx": SWDGE cast-DMA f32->bf16; "castx": engine cast + HWDGE


def _dct_matrix(N):
    n = np.arange(N, dtype=np.float64)
    k = np.arange(N, dtype=np.float64)[:, None]
    d = np.cos(np.pi * (2.0 * n + 1.0) * k / (2.0 * N))
    s = np.where(k == 0, np.sqrt(1.0 / N), np.sqrt(2.0 / N))
    return (d * s).astype(np.float32)  # [N, N], D[k, n]


def _perm_true_channel(g, q):
    return g * 128 + 2 * (q % 64) + (q // 64)


def build_nc(bpc=BPC, repeat=1, variant="full"):
    import concourse.bass as bass
    import concourse.tile as tile
    from concourse import bacc, mybir
    from contextlib import ExitStack

    f32 = mybir.dt.float32
    f32r = mybir.dt.float32r
    bf16 = mybir.dt.bfloat16

    nc = bacc.Bacc("TRN2", target_bir_lowering=False, debug=False)

    xin = nc.dram_tensor("xin", [bpc, C, H, W], f32, kind="ExternalInput").ap()
    dht2_d = nc.dram_tensor("dht2", [128, 64], f32, kind="ExternalInput").ap()
    dwt2_d = nc.dram_tensor("dwt2", [128, 128], f32, kind="ExternalInput").ap()
    w1ps_d = nc.dram_tensor("w1ps", [128, 128], f32, kind="ExternalInput").ap()
    b1_d = nc.dram_tensor("b1v", [1, CH], f32, kind="ExternalInput").ap()
    w2t_d = nc.dram_tensor("w2t", [CH, 256], f32, kind="ExternalInput").ap()
    b2t_d = nc.dram_tensor("b2t", [1, 256], f32, kind="ExternalInput").ap()
    xout = nc.dram_tensor("xout", [bpc, C, H, W], f32, kind="ExternalOutput").ap()

    AF = mybir.ActivationFunctionType

    with tile.TileContext(nc) as tc, ExitStack() as ctx:
        const = ctx.enter_context(tc.tile_pool(name="const", bufs=1))
        xcpool = ctx.enter_context(tc.tile_pool(name="xc", bufs=2))
        xhpool = ctx.enter_context(tc.tile_pool(name="xh", bufs=1))
        scrp = ctx.enter_context(tc.tile_pool(name="scr", bufs=2, space="DRAM"))
        atsb = ctx.enter_context(tc.tile_pool(name="atsb", bufs=2))
        ypool = ctx.enter_context(tc.tile_pool(name="yab", bufs=2))
        flatp = ctx.enter_context(tc.tile_pool(name="flat", bufs=2))
        small = ctx.enter_context(tc.tile_pool(name="small", bufs=2))
        epool = ctx.enter_context(tc.tile_pool(name="energy", bufs=1))
        at_ps = ctx.enter_context(tc.tile_pool(name="atps", bufs=3, space="PSUM"))
        y_ps = ctx.enter_context(tc.tile_pool(name="yps", bufs=2, space="PSUM"))
        s_ps = ctx.enter_context(tc.tile_pool(name="sps", bufs=2, space="PSUM"))

        # ---- constants ----
        dht2_t = const.tile([128, 64], f32)
        nc.sync.dma_start(dht2_t[:], dht2_d[:])
        dwt2_t = const.tile([128, 128], f32)
        nc.sync.dma_start(dwt2_t[:], dwt2_d[:])
        w1ps_t = const.tile([128, 128], f32)
        nc.sync.dma_start(w1ps_t[:], w1ps_d[:])
        b1_t = const.tile([1, CH], f32)
        nc.sync.dma_start(b1_t[:], b1_d[:])
        w2t_t = const.tile([CH, 256], f32)
        nc.sync.dma_start(w2t_t[:], w2t_d[:])
        b2t_t = const.tile([1, 256], f32)
        nc.sync.dma_start(b2t_t[:], b2t_d[:])
        ones_t = const.tile([1, CH], f32)
        nc.vector.memset(ones_t[:], 1.0)
        # DVE-owned bf16 / funneled copies of PE-read constants (the walrus
        # verifier requires matmul operand dtypes to match when f32/f32r is
        # involved, so the DCT matrices go bf16 to pair with bf16 x / A^T)
        dht2r = const.tile([128, 64], f32)
        nc.vector.tensor_copy(dht2r[:], dht2_t[:])
        dwt2r = const.tile([128, 128], f32r)
        nc.vector.tensor_copy(dwt2r[:], dwt2_t[:])
        w1c = const.tile([128, 128], f32)
        nc.vector.tensor_copy(w1c[:], w1ps_t[:])
        b1c = const.tile([1, CH], f32)
        nc.vector.tensor_copy(b1c[:], b1_t[:])
        w2c = const.tile([CH, 256], f32)
        nc.vector.tensor_copy(w2c[:], w2t_t[:])
        b2c = const.tile([1, 256], f32)
        nc.vector.tensor_copy(b2c[:], b2t_t[:])

        energy = [
            epool.tile([128, bpc], f32, tag=f"energy{g}", name=f"energy{g}")
            for g in range(2)
        ]

        def emit_images():
            xh = [None]
            for b in range(bpc):
                pair, b2 = divmod(b, 2)
                half = slice(b2 * 64, b2 * 64 + 64)

                # ---- c-major load: [128 c, (cs, h, w)] ----
                xc = xcpool.tile([128, 8192], f32, tag="xc", name=f"xc{b}")
                nc.sync.dma_start(
                    xc[:].rearrange("c (cs hw) -> c cs hw", cs=2),
                    xin[b].rearrange("(cs c) h w -> c cs (h w)", cs=2),
                )

                if variant == "io":
                    att1 = small.tile([128, 2], f32, tag="att1")
                    nc.vector.memset(att1[:], 1.0)
                    for g in range(2):
                        xseg = xc[:, g * 4096 : (g + 1) * 4096]
                        nc.vector.tensor_scalar_mul(xseg, xseg, att1[:, g : g + 1])
                    nc.scalar.dma_start(
                        xout[b].rearrange("(cs c) h w -> c cs (h w)", cs=2),
                        xc[:].rearrange("c (cs hw) -> c cs hw", cs=2),
                    )
                    continue

                # ---- h-major f32 copy for the DCT path (direct HBM load) ----
                if b2 == 0:
                    xh[0] = xhpool.tile([128, 16384], f32, tag="xh", name=f"xh{pair}")
                xht = xh[0]
                nc.sync.dma_start(
                    xht[half, :].rearrange("h (c w) -> h c w", w=64),
                    xin[b].rearrange("c h w -> h c w"),
                )

                if variant == "noxh":
                    att1 = small.tile([128, 2], f32, tag="att1")
                    nc.vector.memset(att1[:], 1.0)
                    for g in range(2):
                        xseg = xc[:, g * 4096 : (g + 1) * 4096]
                        nc.vector.tensor_scalar_mul(xseg, xseg, att1[:, g : g + 1])
                    nc.scalar.dma_start(
                        xout[b].rearrange("(cs c) h w -> c cs (h w)", cs=2),
                        xc[:].rearrange("c (cs hw) -> c cs hw", cs=2),
                    )
                    continue

                for g in range(2):
                    # ---- M1: A^T for 64 channel-pairs of this group ----
                    at_tiles = []
                    for htile in range(2):
                        at = atsb.tile([128, 2048], f32r, tag="at")
                        at_tiles.append(at)
                        for pc in range(4):
                            aps = at_ps.tile([128, 512], f32, tag="atps")
                            for pp in range(8):
                                p = htile * 32 + pc * 8 + pp
                                c0 = g * 128 + 2 * p
                                nc.tensor.matmul(
                                    aps[:, pp * 64 : (pp + 1) * 64],
                                    lhsT=xht[half, c0 * 64 : (c0 + 2) * 64],
                                    rhs=dht2r[half, :],
                                    start=True,
                                    stop=True,
                                )
                            nc.scalar.copy(
                                at[:, pc * 512 : (pc + 1) * 512], aps[:]
                            )

                    # ---- M2 + |.| eviction ----
                    fl = flatp.tile([128, 4096], bf16, tag="flat")
                    scr = scrp.tile([2, 64, 64, 64], bf16, tag="scr")
                    for htile in range(2):
                        at = at_tiles[htile]
                        yab = ypool.tile([128, 2048], bf16, tag="yab")
                        for chk in range(4):
                            yps = y_ps.tile([128, 512], f32, tag="yps")
                            nc.tensor.matmul(
                                yps[:],
                                lhsT=dwt2r[:],
                                rhs=at[:, chk * 512 : (chk + 1) * 512],
                                start=True,
                                stop=True,
                            )
                            nc.scalar.activation(
                                yab[:, chk * 512 : (chk + 1) * 512], yps[:], AF.Abs
                            )
                        # ---- flatten to channel-major via DRAM bounce ----
                        if variant != "noflat":
                            nc.scalar.dma_start(
                                scr[:, :, htile * 32 : (htile + 1) * 32, :], yab[:]
                            )
                    if variant == "noflat":
                        ecol = energy[g][:, b : b + 1]
                        nc.vector.reduce_sum(
                            out=ecol, in_=yab[:, 0:4], axis=mybir.AxisListType.X
                        )
                    else:
                        for m in range(2):
                            nc.scalar.dma_start(
                                fl[m * 64 : (m + 1) * 64, :].rearrange(
                                    "p (j i) -> p j i", j=64
                                ),
                                scr[m].rearrange("j p i -> p j i"),
                            )
                        # ---- top-k energy ----
                        t8 = small.tile([128, 8], bf16, tag="top8")
                        nc.vector.max(out=t8[:], in_=fl[:, 1:4096])
                        ecol = energy[g][:, b : b + 1]
                        nc.vector.reduce_sum(
                            out=ecol, in_=t8[:, 0:4], axis=mybir.AxisListType.X
                        )
                        dc32 = small.tile([128, 1], f32, tag="dc32")
                        nc.vector.tensor_copy(dc32[:], fl[:, 0:1])
                        nc.vector.tensor_add(ecol, ecol, dc32[:])

                # ---- MLP (per image) ----
                hps = s_ps.tile([CH, 1], f32, tag="sps")
                nc.tensor.matmul(
                    hps[:], lhsT=w1c[:, 0:CH], rhs=energy[0][:, b : b + 1],
                    start=True, stop=False,
                )
                nc.tensor.matmul(
                    hps[:], lhsT=w1c[:, CH : 2 * CH], rhs=energy[1][:, b : b + 1],
                    start=False, stop=False,
                )
                nc.tensor.matmul(
                    hps[:], lhsT=b1c[:], rhs=ones_t[:, 0:1], start=False, stop=True
                )
                hid = small.tile([CH, 1], f32, tag="hid")
                nc.scalar.activation(hid[:], hps[:], AF.Relu)

                att = small.tile([128, 2], f32, tag="att")
                for g in range(2):
                    aps2 = s_ps.tile([128, 1], f32, tag="sps")
                    nc.tensor.matmul(
                        aps2[:], lhsT=w2c[:, g * 128 : (g + 1) * 128], rhs=hid[:],
                        start=True, stop=False,
                    )
                    nc.tensor.matmul(
                        aps2[:], lhsT=b2c[:, g * 128 : (g + 1) * 128],
                        rhs=ones_t[:, 0:1], start=False, stop=True,
                    )
                    nc.scalar.activation(att[:, g : g + 1], aps2[:], AF.Sigmoid)

                # ---- multiply (per-partition scalar, true channel order) ----
                for g in range(2):
                    xseg = xc[:, g * 4096 : (g + 1) * 4096]
                    nc.vector.tensor_scalar_mul(xseg, xseg, att[:, g : g + 1])

                # ---- store ----
                nc.scalar.dma_start(
                    xout[b].rearrange("(cs c) h w -> c cs (h w)", cs=2),
                    xc[:].rearrange("c (cs hw) -> c cs hw", cs=2),
                )

        if repeat > 1:
            with tc.For_i(0, repeat, 1):
                emit_images()
        else:
            emit_images()

    nc.compile()
    return nc


def make_host_inputs():
    DH = _dct_matrix(H)
    DW = _dct_matrix(W)
    dht2 = np.zeros((128, 64), np.float32)
    dht2[0:64, :] = DH.T
    dht2[64:128, :] = DH.T
    dwt2 = np.zeros((128, 128), np.float32)
    dwt2[0:64, 0:64] = DW.T
    dwt2[64:128, 64:128] = DW.T
    return dht2, dwt2


def make_weight_inputs(w1, b1, w2, b2):
    w1ps = np.zeros((128, 128), np.float32)
    for g in range(2):
        cs = np.array([_perm_true_channel(g, q) for q in range(128)])
        w1ps[:, g * CH : (g + 1) * CH] = w1[cs, :]
    b1v = b1.reshape(1, CH).astype(np.float32)
    w2t = np.ascontiguousarray(w2, dtype=np.float32)
    b2t = b2.reshape(1, 256).astype(np.float32)
    return w1ps, b1v, w2t, b2t


_CACHE = {}


def _get_runner(repeat=1, variant="full"):
    """Build (once) a cached jitted SPMD executable over 8 cores."""
    key = ("runner", repeat, variant)
    if key in _CACHE:
        return _CACHE[key]
    import jax
    from jax.experimental.shard_map import shard_map
    from jax.sharding import Mesh, PartitionSpec
    from concourse import bass2jax, mybir
    from concourse.bass2jax import _bass_exec_p, install_neuronx_cc_hook

    install_neuronx_cc_hook()
    nc = build_nc(BPC, repeat=repeat, variant=variant)

    partition_name = (
        nc.partition_id_tensor.name if nc.partition_id_tensor else None
    )
    in_names, out_names, out_avals = [], [], []
    for alloc in nc.m.functions[0].allocations:
        if not isinstance(alloc, mybir.MemoryLocationSet):
            continue
        name = alloc.memorylocations[0].name
        if alloc.kind == "ExternalInput":
            if name != partition_name:
                in_names.append(name)
        elif alloc.kind == "ExternalOutput":
            out_names.append(name)
            out_avals.append(
                jax.core.ShapedArray(
                    tuple(alloc.tensor_shape), mybir.dt.np(alloc.dtype)
                )
            )
    n_params = len(in_names)
    all_in_names = in_names + out_names
    if partition_name is not None:
        all_in_names = all_in_names + [partition_name]

    def _body(*args):
        operands = list(args)
        if partition_name is not None:
            operands.append(bass2jax.partition_id_tensor())
        outs = _bass_exec_p.bind(
            *operands,
            out_avals=tuple(out_avals),
            in_names=tuple(all_in_names),
            out_names=tuple(out_names),
            lowering_input_output_aliases=(),
            sim_require_finite=True,
            sim_require_nnan=True,
            nc=nc,
        )
        return tuple(outs)

    devices = jax.devices()[:NCORES]
    mesh = Mesh(np.asarray(devices), ("core",))
    nin = n_params + len(out_names)
    sharded = jax.jit(
        shard_map(
            _body,
            mesh=mesh,
            in_specs=(PartitionSpec("core"),) * nin,
            out_specs=(PartitionSpec("core"),) * len(out_names),
            check_rep=False,
        ),
        donate_argnums=tuple(range(n_params, nin)),
        keep_unused=True,
    )
    runner = (sharded, in_names, out_names, out_avals)
    _CACHE[key] = runner
    return runner


def make_concat_inputs(x, w1, b1, w2, b2):
    """Per-core inputs concatenated along axis 0 (shard_map layout)."""
    x = np.asarray(x, dtype=np.float32)
    dht2, dwt2 = make_host_inputs()
    w1ps, b1v, w2t, b2t = make_weight_inputs(
        np.asarray(w1, np.float32),
        np.asarray(b1, np.float32),
        np.asarray(w2, np.float32),
        np.asarray(b2, np.float32),
    )
    per_core = {
        "dht2": dht2, "dwt2": dwt2, "w1ps": w1ps, "b1v": b1v,
        "w2t": w2t, "b2t": b2t,
    }
    vals = {"xin": np.ascontiguousarray(x)}
    for k, v in per_core.items():
        vals[k] = np.concatenate([v] * NCORES, axis=0)
    return vals


def kernel(x, w1, b1, w2, b2):
    sharded, in_names, out_names, out_avals = _get_runner()
    vals = make_concat_inputs(x, w1, b1, w2, b2)
    concat_in = [vals[n] for n in in_names]
    concat_zeros = [
        np.zeros((NCORES * a.shape[0], *a.shape[1:]), a.dtype) for a in out_avals
    ]
    out_arrs = sharded(*concat_in, *concat_zeros)
    return np.asarray(out_arrs[out_names.index("xout")]).astype(np.float32)
